# revision 7
# baseline (speedup 1.0000x reference)
"""Trainium2 Bass kernel for nn_GatDot_rel (GAT-style dot-product attention
with relation bias), data-parallel over the batch dim across 8 NeuronCores.

Math (reference):
    rel   = rel_emb[s_mask]                      # (B,N,D)
    q     = Q @ W1.T + b1                        # (B,D)
    k     = K @ W2.T + b2                        # (B,N,D)
    y     = rel @ W3.T + b3                      # (B,N,1)
    alpha = einsum(k, q) + y - (1-adj)*1e30      # (B,N)
    w     = softmax(alpha, -1)                   # (B,1,N)
    out   = einsum(w, V)                         # (B,D)

Key algebraic fold: alpha[b,n] = K[b,n,:]·(q[b]@W2) + q[b]·b2 + y[b,n],
so the (B,N,D) projected k tensor is never materialized.  The tiny
per-batch terms (q@W2, q·b2, y, mask) are computed on host; the device
streams K and V once each (the irreducible ~1 GB of HBM traffic) doing:

    alpha[b,n] = K[b,n,:]·q2[b] + bias[b,n]      (DVE fused mult+reduce)
    e = exp(alpha); w = e / sum(e)               (ACT exp+rowsum, PE allsum)
    out[b,:]  = sum_n w[b,n] * V[b,n,:]          (PE rank-1 accumulated MMs)

Per-core layout: n = p*16 + j  (p = SBUF partition, j = free chunk), which
is the natural C-order reshape of (2048, 256) -> (128, 16, 256).
"""

import numpy as np
from contextlib import ExitStack

import concourse.bass as bass
import concourse.bacc as bacc
import concourse.mybir as mybir
import concourse.tile as tile
from concourse.bass_utils import run_bass_kernel_spmd

NCORES = 8
B, N, D = 256, 2048, 256
BLOC = B // NCORES  # 32 batch rows per core
P = 128             # SBUF partitions
J = N // P          # 16 free-dim chunks per batch row
F32 = mybir.dt.float32


def build_nc(bloc: int = BLOC, kbufs: int = 3, vbufs: int = 3,
             alpha_mode: str = "stt") -> bass.Bass:
    # Bacc (not raw Bass): its nop-fusion/sem legalization splits the
    # multi-wait sync_info Tile emits into 1-wait-per-instruction chains,
    # which this walrus build requires.
    nc = bacc.Bacc("TRN2", target_bir_lowering=False, debug=False)
    Kt = nc.declare_dram_parameter("Kt", [bloc, P, J, D], F32, isOutput=False)
    Vt = nc.declare_dram_parameter("Vt", [bloc, P, J, D], F32, isOutput=False)
    q2b = nc.declare_dram_parameter("q2b", [P, bloc, D], F32, isOutput=False)
    biasP = nc.declare_dram_parameter("biasP", [P, bloc, J], F32, isOutput=False)
    wP = nc.declare_dram_parameter("attn_w", [P, bloc * J], F32, isOutput=True)
    sP = nc.declare_dram_parameter("attn_s", [1, bloc * D], F32, isOutput=True)

    with ExitStack() as ctx:
        tc = ctx.enter_context(tile.TileContext(nc))
        const = ctx.enter_context(tc.tile_pool(name="const", bufs=1))
        kpool = ctx.enter_context(tc.tile_pool(name="kpool", bufs=kbufs))
        vpool = ctx.enter_context(tc.tile_pool(name="vpool", bufs=vbufs))
        small = ctx.enter_context(tc.tile_pool(name="small", bufs=4))
        scratch = ctx.enter_context(tc.tile_pool(name="scratch", bufs=2))
        psA = ctx.enter_context(tc.tile_pool(name="psA", bufs=2, space="PSUM"))
        psB = ctx.enter_context(tc.tile_pool(name="psB", bufs=2, space="PSUM"))

        q2b_t = const.tile([P, bloc, D], F32)
        nc.sync.dma_start(q2b_t[:], q2b[:])
        bias_t = const.tile([P, bloc, J], F32)
        nc.sync.dma_start(bias_t[:], biasP[:])
        ones_t = const.tile([P, P], F32)
        nc.vector.memset(ones_t[:], 1.0)
        w_all = const.tile([P, bloc * J], F32)
        s_all = const.tile([1, bloc * D], F32)

        for b in range(bloc):
            kt = kpool.tile([P, J, D], F32)
            nc.sync.dma_start(kt[:], Kt[b])

            # alpha[:, j] = sum_d kt[:, j, d] * q2[:, d]  + bias[:, b, j]
            alpha_t = small.tile([P, J], F32)
            if alpha_mode == "stt":
                for j in range(J):
                    prod = scratch.tile([P, D], F32)
                    nc.vector.scalar_tensor_tensor(
                        out=prod[:],
                        in0=kt[:, j, :],
                        scalar=1.0,
                        in1=q2b_t[:, b, :],
                        op0=mybir.AluOpType.mult,
                        op1=mybir.AluOpType.mult,
                        accum_out=alpha_t[:, j : j + 1],
                    )
                nc.vector.tensor_add(
                    alpha_t[:], alpha_t[:], bias_t[:, b, :])
            elif alpha_mode == "ttr":
                for j in range(J):
                    prod = scratch.tile([P, D], F32)
                    nc.vector.tensor_tensor_reduce(
                        out=prod[:],
                        in0=kt[:, j, :],
                        in1=q2b_t[:, b, :],
                        scale=1.0,
                        scalar=bias_t[:, b, j : j + 1],
                        op0=mybir.AluOpType.mult,
                        op1=mybir.AluOpType.add,
                        accum_out=alpha_t[:, j : j + 1],
                    )
            else:  # mulred: in-place product into the K tile, then reduce
                nc.vector.tensor_mul(
                    kt[:], kt[:],
                    q2b_t[:, b : b + 1, :].broadcast_to((P, J, D)))
                nc.vector.tensor_reduce(
                    alpha_t[:], kt[:], axis=mybir.AxisListType.X,
                    op=mybir.AluOpType.add)
                nc.vector.tensor_add(
                    alpha_t[:], alpha_t[:], bias_t[:, b, :])

            # e = exp(alpha), part = per-partition row sum of e
            e_t = small.tile([P, J], F32)
            part_t = small.tile([P, 1], F32)
            nc.scalar.activation(
                e_t[:], alpha_t[:], mybir.ActivationFunctionType.Exp,
                accum_out=part_t[:],
            )

            # total = all-partition sum, broadcast to all partitions via
            # ones(128,128).T @ part  (PE), then w = e * (1/total)
            tot = psA.tile([P, 1], F32)
            nc.tensor.matmul(tot[:], ones_t[:], part_t[:], start=True, stop=True)
            inv_t = small.tile([P, 1], F32)
            nc.vector.reciprocal(inv_t[:], tot[:])
            nc.vector.tensor_scalar_mul(
                w_all[:, b * J : (b + 1) * J], e_t[:], inv_t[:]
            )

            # attn_sum[b] = sum_j  w[:, j].T @ V[:, j, :]   (PSUM accumulate)
            vt = vpool.tile([P, J, D], F32)
            nc.sync.dma_start(vt[:], Vt[b])
            sp = psB.tile([1, D], F32)
            for j in range(J):
                nc.tensor.matmul(
                    sp[:],
                    w_all[:, b * J + j : b * J + j + 1],
                    vt[:, j, :],
                    start=(j == 0),
                    stop=(j == J - 1),
                )
            nc.scalar.copy(s_all[:, b * D : (b + 1) * D], sp[:])

        nc.sync.dma_start(wP[:], w_all[:])
        nc.sync.dma_start(sP[:], s_all[:])
    return nc


_NC_CACHE: dict = {}


def _get_nc() -> bass.Bass:
    if "nc" not in _NC_CACHE:
        nc = build_nc()
        nc.finalize()  # Bacc.finalize runs the bacc compile pipeline
        _NC_CACHE["nc"] = nc
    return _NC_CACHE["nc"]


def host_precompute(Q, adj, s_mask, W1, b1, W2, b2, W3, b3, rel_emb):
    """Tiny O(B*D^2 + B*N) host-side linear algebra, in f64 for accuracy."""
    q = Q.astype(np.float64) @ W1.astype(np.float64).T + b1.astype(np.float64)
    q2 = (q @ W2.astype(np.float64)).astype(np.float32)          # (B, D)
    c = (q @ b2.astype(np.float64)).astype(np.float32)           # (B,)
    yv = (rel_emb.astype(np.float64) @ W3[0].astype(np.float64)
          + b3.astype(np.float64)[0]).astype(np.float32)         # (2,)
    y = yv[np.asarray(s_mask).astype(np.int64)]                  # (B, N)
    bias = (c[:, None] + y).astype(np.float32)
    bias = bias - (1.0 - adj.astype(np.float32)) * np.float32(1e30)
    return q2, bias.astype(np.float32)


def make_in_maps(K, V, q2, bias):
    in_maps = []
    for cix in range(NCORES):
        sl = slice(cix * BLOC, (cix + 1) * BLOC)
        Ktc = np.ascontiguousarray(K[sl]).reshape(BLOC, P, J, D)
        Vtc = np.ascontiguousarray(V[sl]).reshape(BLOC, P, J, D)
        q2c = np.ascontiguousarray(
            np.broadcast_to(q2[sl].reshape(1, BLOC, D), (P, BLOC, D))
        )
        biasc = np.ascontiguousarray(
            bias[sl].reshape(BLOC, P, J).transpose(1, 0, 2)
        )
        in_maps.append({"Kt": Ktc, "Vt": Vtc, "q2b": q2c, "biasP": biasc})
    return in_maps


def unshard(results):
    w_parts, s_parts = [], []
    for cix in range(NCORES):
        wPc = np.asarray(results[cix]["attn_w"]).reshape(P, BLOC, J)
        w_parts.append(wPc.transpose(1, 0, 2).reshape(BLOC, N))
        s_parts.append(np.asarray(results[cix]["attn_s"]).reshape(BLOC, D))
    attn_weight = np.concatenate(w_parts)[:, None, :].astype(np.float32)
    attn_sum = np.concatenate(s_parts).astype(np.float32)
    return attn_weight, attn_sum


def kernel(Q, K, V, adj, s_mask, W1, b1, W2, b2, W3, b3, rel_emb):
    Q = np.asarray(Q, np.float32)
    K = np.asarray(K, np.float32)
    V = np.asarray(V, np.float32)
    adj = np.asarray(adj, np.float32)
    s_mask = np.asarray(s_mask)
    q2, bias = host_precompute(
        Q, adj, s_mask,
        np.asarray(W1), np.asarray(b1), np.asarray(W2), np.asarray(b2),
        np.asarray(W3), np.asarray(b3), np.asarray(rel_emb),
    )
    in_maps = make_in_maps(K, V, q2, bias)
    res = run_bass_kernel_spmd(_get_nc(), in_maps, list(range(NCORES))).results
    return unshard(res)
